# revision 4
# baseline (speedup 1.0000x reference)
"""Trainium2 Bass kernel for nn_CrossAttUnit (ragged cross-attention unit).

Math (per 64-token segment, N=262144 tokens total, H=256, D=64):
    yk = y_seg @ k            [64, 64]
    yq = yhat_seg @ q         [64, 64]
    M  = (yk @ yq.T) / 8      [64, 64]
    attn = softmax(M, axis=1) + 1e-6      (row softmax)
    W  = attn / attn.sum(axis=0)          (column normalize)
Output: [4096, 64, 64] float32.

Sharding: data-parallel over segments; core i handles tokens
[i*32768, (i+1)*32768) (512 whole segments). k, q replicated.

v2 changes vs v1 baseline:
  - 1024-token groups (8 tiles of 128; 32 groups/core): bigger DMAs (1MB),
    fewer + larger elementwise/matmul ops.
  - Projections fused across 4 token-tiles: moving operand [128, 512]
    (max fp32 free), 2 K-chunks accumulated -> 4 matmuls per half-group
    instead of 16 per group.
  - Column-sum matmul with free=512.
  - gpsimd only touches SBUF (A-pass, W-pass); all PSUM->SBUF copies split
    across DVE/ACT.
  - rowmax via tensor_reduce(negate=True) feeding exp bias; 1/colsum via a
    single custom-DVE approximate reciprocal (no ACT table switching).
"""

import numpy as np

N_TOTAL = 262144
H = 256
D = 64
L = 64  # seg_len
NCORES = 8
N_LOC = N_TOTAL // NCORES  # 32768
TILE_TOK = 128
GROUP_TILES = 8
GROUP_TOK = TILE_TOK * GROUP_TILES  # 1024
HALF_TILES = GROUP_TILES // 2
SCALE = 0.125  # 1/sqrt(D)
EPS = 1e-6

# fp32r (TF32-class) matmuls measured rel err 0.35 on HW -- the softmax
# exponent amplifies any rounding of y/k/q into O(10%) weight errors, so the
# whole y->M chain must stay true fp32.
PROJ_F32R = False

_CACHE = {}


def _build_program(n_loc, proj_f32r=PROJ_F32R):
    import concourse.bacc as bacc
    import concourse.tile as tile
    from concourse import mybir

    f32 = mybir.dt.float32
    f32r = mybir.dt.float32r
    FT = mybir.ActivationFunctionType
    OP = mybir.AluOpType

    fin = f32r if proj_f32r else f32  # input-side matmul operand dtype

    nc = bacc.Bacc("TRN2", target_bir_lowering=False)

    y_d = nc.dram_tensor("y", [n_loc, H], fin, kind="ExternalInput")
    yh_d = nc.dram_tensor("yh", [n_loc, H], fin, kind="ExternalInput")
    # k/q zero-padded to [H, 128] on host (cols D..127 zero): projection
    # outputs are [128, tok] with upper 64 partitions zero, so every matmul
    # contracts a full K=128 at row base 0 (partial-row stationary loads with
    # nonzero partition base hard-fail on HW).
    k_d = nc.dram_tensor("k", [H, 128], fin, kind="ExternalInput")
    q_d = nc.dram_tensor("q", [H, 128], fin, kind="ExternalInput")
    id_d = nc.dram_tensor("ident", [128, 128], fin, kind="ExternalInput")
    bd_d = nc.dram_tensor("bdiag", [128, 128], f32, kind="ExternalInput")
    w_d = nc.dram_tensor("w", [n_loc, L], f32, kind="ExternalOutput")

    ngroups = n_loc // GROUP_TOK

    with tile.TileContext(nc) as tc:
        with (
            tc.tile_pool(name="consts", bufs=1) as consts,
            tc.tile_pool(name="ld", bufs=4) as ldp,
            tc.tile_pool(name="yT", bufs=2) as yTp,
            tc.tile_pool(name="pkq", bufs=3) as pkqp,
            tc.tile_pool(name="soft", bufs=2) as softp,
            tc.tile_pool(name="wout", bufs=3) as woutp,
            tc.tile_pool(name="stats", bufs=4) as statp,
            tc.tile_pool(name="ps_t", bufs=3, space="PSUM") as ps_t,
            tc.tile_pool(name="ps_p", bufs=1, space="PSUM") as ps_p,
            tc.tile_pool(name="ps_m", bufs=2, space="PSUM") as ps_m,
            tc.tile_pool(name="ps_c", bufs=1, space="PSUM") as ps_c,
        ):
            k_sb = consts.tile([128, 2, 128], fin)
            q_sb = consts.tile([128, 2, 128], fin)
            id_sb = consts.tile([128, 128], fin)
            bd_sb = consts.tile([128, 128], f32)
            nc.sync.dma_start(out=k_sb[:], in_=k_d.rearrange("(c p) d -> p c d", p=128))
            nc.sync.dma_start(out=q_sb[:], in_=q_d.rearrange("(c p) d -> p c d", p=128))
            nc.sync.dma_start(out=id_sb[:], in_=id_d[:])
            nc.sync.dma_start(out=bd_sb[:], in_=bd_d[:])

            # Software-pipelined issue order. Loads are emitted 2 groups ahead
            # (y on SP/HWDGE, yh on Pool/SWDGE -- one ring can't sustain both
            # 1MB loads per group); out-DMAs are delayed one group so a W_sb
            # wait never blocks prefetch at the head of the SP queue.
            pending_out = []
            ld_tiles = {}

            def emit_loads(gg):
                if gg >= ngroups:
                    return
                r0 = gg * GROUP_TOK
                y_sb = ldp.tile([128, GROUP_TILES, H], fin, tag="y")
                yh_sb = ldp.tile([128, GROUP_TILES, H], fin, tag="yh")
                nc.sync.dma_start(
                    out=y_sb[:],
                    in_=y_d[r0 : r0 + GROUP_TOK, :].rearrange("(t p) h -> p t h", p=128),
                )
                nc.gpsimd.dma_start(
                    out=yh_sb[:],
                    in_=yh_d[r0 : r0 + GROUP_TOK, :].rearrange(
                        "(t p) h -> p t h", p=128
                    ),
                )
                ld_tiles[gg] = (y_sb, yh_sb)

            def flush_out():
                for W_sb_p, row0_p in pending_out:
                    nc.sync.dma_start(
                        out=w_d[row0_p : row0_p + GROUP_TOK, :].rearrange(
                            "(t p) m -> p t m", p=128
                        ),
                        in_=W_sb_p[:],
                    )
                pending_out.clear()

            emit_loads(0)
            emit_loads(1)
            for g in range(ngroups):
                row0 = g * GROUP_TOK
                emit_loads(g + 2)
                flush_out()
                y_sb, yh_sb = ld_tiles.pop(g)

                # ---- transposes + projections, per half-group of 4 tiles ----
                ykq_sbs = []
                for hg in range(2):
                    # yT_sb free layout: [tile(4), tensor(2), chunk(2), tok(128)]
                    # -- per-transpose-tile copies land contiguous (keeps the
                    # DVE in 2x mode); the strided view is pushed onto the PE
                    # moving-operand AP, which is insensitive to strides.
                    yT_sb = yTp.tile([128, 4, 2, 2, TILE_TOK], fin)
                    for tl in range(HALF_TILES):
                        t = hg * HALF_TILES + tl
                        yT_ps = ps_t.tile([128, 4, TILE_TOK], fin)
                        nc.tensor.transpose(
                            yT_ps[:, 0, :],
                            y_sb[:, t, 0:128],
                            id_sb[:],
                        )
                        nc.tensor.transpose(
                            yT_ps[:, 1, :],
                            y_sb[:, t, 128:256],
                            id_sb[:],
                        )
                        nc.tensor.transpose(
                            yT_ps[:, 2, :],
                            yh_sb[:, t, 0:128],
                            id_sb[:],
                        )
                        nc.tensor.transpose(
                            yT_ps[:, 3, :],
                            yh_sb[:, t, 128:256],
                            id_sb[:],
                        )
                        dst = yT_sb[:, tl, :, :, :]
                        # 6 of 8 yT copies on DVE, 2 on ACT (ACT also owns the
                        # 8 exps; DVE is cheaper per copy)
                        if tl == 1:
                            nc.scalar.copy(dst, yT_ps[:])
                        else:
                            nc.vector.tensor_copy(dst, yT_ps[:])
                    # fused projections over 512 tokens: ykT = k_pad^T @ y^T
                    ykq_sb = pkqp.tile([128, 2, 4 * TILE_TOK], f32)
                    yk_ps = ps_p.tile([128, 4 * TILE_TOK], f32, tag="yk")
                    for c in range(2):
                        nc.tensor.matmul(
                            yk_ps[:],
                            k_sb[:, c, :],
                            yT_sb[:, :, 0, c, :],
                            start=(c == 0),
                            stop=(c == 1),
                        )
                    nc.vector.tensor_copy(ykq_sb[:, 0, :], yk_ps[:])
                    yq_ps = ps_p.tile([128, 4 * TILE_TOK], f32, tag="yq")
                    for c in range(2):
                        nc.tensor.matmul(
                            yq_ps[:],
                            q_sb[:, c, :],
                            yT_sb[:, :, 1, c, :],
                            start=(c == 0),
                            stop=(c == 1),
                        )
                    nc.scalar.copy(ykq_sb[:, 1, :], yq_ps[:])
                    ykq_sbs.append(ykq_sb)

                # ---- per-segment M = yk_seg @ yq_seg^T (K=128, zero upper) ----
                M_ps = ps_m.tile([128, GROUP_TILES, L], f32)
                for t in range(GROUP_TILES):
                    ykq_sb = ykq_sbs[t // HALF_TILES]
                    o = (t % HALF_TILES) * TILE_TOK
                    for s in range(2):
                        nc.tensor.matmul(
                            M_ps[s * 64 : (s + 1) * 64, t, :],
                            ykq_sb[:, 0, o + s * 64 : o + (s + 1) * 64],
                            ykq_sb[:, 1, o + s * 64 : o + (s + 1) * 64],
                            start=True,
                            stop=True,
                        )

                # ---- row softmax over free axis (m) ----
                nmax = statp.tile([128, GROUP_TILES], f32, tag="nmax")
                nbias = statp.tile([128, GROUP_TILES], f32, tag="nbias")
                rowsum = statp.tile([128, GROUP_TILES], f32, tag="rowsum")
                rr = statp.tile([128, GROUP_TILES], f32, tag="rr")
                nc.vector.tensor_reduce(
                    nmax[:], M_ps[:], axis=mybir.AxisListType.X, op=OP.max, negate=True
                )
                nc.vector.tensor_scalar_mul(nbias[:], nmax[:], SCALE)
                A_sb = softp.tile([128, GROUP_TILES, L], f32, tag="A")
                for t in range(GROUP_TILES):
                    nc.scalar.activation(
                        A_sb[:, t, :],
                        M_ps[:, t, :],
                        FT.Exp,
                        bias=nbias[:, t : t + 1],
                        scale=SCALE,
                        accum_out=rowsum[:, t : t + 1],
                    )
                nc.vector.reciprocal(rr[:], rowsum[:])
                # attn' = E * (1/rowsum) + EPS  (SBUF-only -> gpsimd)
                for t in range(GROUP_TILES):
                    nc.gpsimd.tensor_scalar(
                        A_sb[:, t, :],
                        A_sb[:, t, :],
                        rr[:, t : t + 1],
                        EPS,
                        op0=OP.mult,
                        op1=OP.add,
                    )
                # per-segment column sums broadcast to all 64 rows
                CS_ps = ps_c.tile([128, GROUP_TILES, L], f32)
                nc.tensor.matmul(
                    CS_ps[:], bd_sb[:], A_sb[:], start=True, stop=True
                )
                # rc = 1/CS: single custom-DVE op (~18 correct bits, plenty for
                # 2e-2 tol; avoids ACT Ln, whose table set differs from Exp's
                # and forced 2 x 1.3us table reloads per group)
                rc_sb = softp.tile([128, GROUP_TILES, L], f32, tag="rc")
                nc.vector.reciprocal_approx_fast(out=rc_sb[:], in_=CS_ps[:])
                W_sb = woutp.tile([128, GROUP_TILES, L], f32)
                nc.gpsimd.tensor_tensor(W_sb[:], A_sb[:], rc_sb[:], op=OP.mult)
                pending_out.append((W_sb, row0))
            flush_out()

    nc.compile()
    return nc


def _consts():
    ident = np.eye(128, dtype=np.float32)
    bdiag = np.zeros((128, 128), dtype=np.float32)
    bdiag[:64, :64] = 1.0
    bdiag[64:, 64:] = 1.0
    return ident, bdiag


def _pad_proj(m):
    """[H, D] -> [H, 128] with zero right half."""
    out = np.zeros((H, 128), dtype=np.float32)
    out[:, :D] = np.asarray(m, dtype=np.float32)
    return out


def _get_program(n_loc):
    if n_loc not in _CACHE:
        _CACHE[n_loc] = _build_program(n_loc)
    return _CACHE[n_loc]


def _in_maps(yhat_embedding, y_embedding, k, q):
    ident, bdiag = _consts()
    y = np.ascontiguousarray(np.asarray(y_embedding, dtype=np.float32))
    yh = np.ascontiguousarray(np.asarray(yhat_embedding, dtype=np.float32))
    kk = _pad_proj(k)
    qq = _pad_proj(q)
    in_maps = []
    for i in range(NCORES):
        sl = slice(i * N_LOC, (i + 1) * N_LOC)
        in_maps.append(
            {
                "y": y[sl],
                "yh": yh[sl],
                "k": kk,
                "q": qq,
                "ident": ident,
                "bdiag": bdiag,
            }
        )
    return in_maps


def _run(yhat_embedding, y_embedding, k, q, trace=False):
    from concourse.bass_utils import run_bass_kernel_spmd

    nc = _get_program(N_LOC)
    in_maps = _in_maps(yhat_embedding, y_embedding, k, q)
    res = run_bass_kernel_spmd(
        nc, in_maps, core_ids=list(range(NCORES)), trace=trace
    )
    w = np.concatenate([r["w"] for r in res.results], axis=0)
    out = w.reshape(N_TOTAL // L, L, L)
    return out, res


def kernel(**inputs):
    yhat_embedding = inputs["yhat_embedding"]
    y_embedding = inputs["y_embedding"]
    k = inputs["k"]
    q = inputs["q"]
    seg_len = int(inputs.get("seg_len", L))
    assert seg_len == L, f"kernel hardcodes seg_len={L}, got {seg_len}"
    out, _ = _run(yhat_embedding, y_embedding, k, q, trace=False)
    return out
